# revision 18
# baseline (speedup 1.0000x reference)
"""Trainium2 Bass kernel for nn_ChannelWiseAttention (topk_masking).

Math (KDIM=1 makes attention scores a rank-1 outer product):
  q = x @ Wq + bq            [B, C]
  kk = x @ Wk + bk           [B, C]
  S[b,c,m]   = q[b,c] * kk[b,m]                  (scale = 1/sqrt(1) = 1)
  W          = softmax(S, axis=-1)
  imp[b,m]   = mean_c W[b,c,m]  (ranking-equivalent: sum_c exp(q_c kk_m)/Z_c)
  topk(imp)  -> indices (sorted by value desc), gather rows of x.

Device plan (pure data parallel: B=32 sharded 4 per core across 8 cores):
  Launch 1 (per core, per batch):
    - DMA x[b] in [128, 16, 512] layout (channel c = t*128 + p).
    - DVE: q/kk via fused multiply+accumulate against broadcast Wq/Wk rows.
    - PE transpose + SBUF DMA + GpSimd partition_broadcast to build a
      [128, 2048] broadcast of kk.
    - ACT: one instruction per 128-channel tile fuses the outer product,
      exp, and row-sum:  E_t = exp(kk_bcast * q_p), accum_out -> Z.
      (No row-max subtraction needed: |S| <= ~25, exp stays in fp32 range.)
    - DVE reciprocal r = 1/Z; PE fp32 matmuls accumulate imp = sum_t r_t^T E_t.
  Host: stable argsort of imp (matches jax.lax.top_k tie-breaking).
  Launch 2 (per core, per batch): GpSimd dma_gather of the selected 614 rows
    (padded to 640) from x in HBM, then DMA to the output.
"""

import numpy as np
from contextlib import ExitStack

import concourse.bass as bass
import concourse.bacc as bacc_mod
import concourse.mybir as mybir
from concourse.tile import TileContext
from concourse.bass_utils import run_bass_kernel_spmd
from concourse.masks import make_identity

F32 = mybir.dt.float32
I16 = mybir.dt.int16

B, C, D = 32, 2048, 512
NCORES = 8
BPC = B // NCORES          # batches per core
NT = C // 128              # 16 channel tiles per batch
K = max(1, int(C * 0.3))   # 614
KPAD = 640                 # next multiple of 128
GD = KPAD // 128           # 5


def build_imp_kernel(repeat=1):
    nc = bacc_mod.Bacc("TRN2", target_bir_lowering=False)
    x = nc.declare_dram_parameter("x", [BPC, C, D], F32, isOutput=False)
    wq = nc.declare_dram_parameter("wq", [D], F32, isOutput=False)
    wk = nc.declare_dram_parameter("wk", [D], F32, isOutput=False)
    bqk = nc.declare_dram_parameter("bqk", [2], F32, isOutput=False)
    imp = nc.declare_dram_parameter("imp", [BPC, C], F32, isOutput=True)

    with TileContext(nc) as tc, ExitStack() as ctx:
        const = ctx.enter_context(tc.tile_pool(name="const", bufs=1))
        xpool = ctx.enter_context(tc.tile_pool(name="xpool", bufs=2))
        epool = ctx.enter_context(tc.tile_pool(name="epool", bufs=3))
        kkpool = ctx.enter_context(tc.tile_pool(name="kkpool", bufs=2))
        qk = ctx.enter_context(tc.tile_pool(name="qk", bufs=2))
        small = ctx.enter_context(tc.tile_pool(name="small", bufs=2))
        pimp = ctx.enter_context(tc.tile_pool(name="pimp", bufs=1, space="PSUM"))
        ptr = ctx.enter_context(tc.tile_pool(name="ptr", bufs=2, space="PSUM"))

        # Broadcast W rows across all partitions (DMA partition-stride 0).
        def bcast_ap(handle, offset, n):
            a = handle.ap()
            return bass.AP(tensor=a.tensor, offset=offset, ap=[[0, 128], [1, n]])

        wq_bc = const.tile([128, D], F32)
        nc.gpsimd.dma_start(out=wq_bc, in_=bcast_ap(wq, 0, D))
        wk_bc = const.tile([128, D], F32)
        nc.gpsimd.dma_start(out=wk_bc, in_=bcast_ap(wk, 0, D))
        bq_bc = const.tile([128, 1], F32)
        nc.gpsimd.dma_start(out=bq_bc, in_=bcast_ap(bqk, 0, 1))
        bk_bc = const.tile([128, 1], F32)
        nc.gpsimd.dma_start(out=bk_bc, in_=bcast_ap(bqk, 1, 1))
        identity = const.tile([128, 128], F32)
        make_identity(nc, identity)

        for b in range(BPC * repeat):
            b = b % BPC
            x_t = xpool.tile([128, NT, D], F32)
            nc.sync.dma_start(out=x_t, in_=x.ap()[b].rearrange("(t p) d -> p t d", p=128))

            qcol = qk.tile([128, NT], F32)
            kcol = qk.tile([128, NT], F32)
            zcol = qk.tile([128, NT], F32)
            rcol = qk.tile([128, NT], F32)
            scratch = qk.tile([128, D], F32)
            for t in range(NT):
                nc.vector.scalar_tensor_tensor(
                    out=scratch, in0=x_t[:, t, :], scalar=0.0, in1=wq_bc,
                    op0=mybir.AluOpType.bypass, op1=mybir.AluOpType.mult,
                    accum_out=qcol[:, t:t + 1])
                nc.vector.scalar_tensor_tensor(
                    out=scratch, in0=x_t[:, t, :], scalar=0.0, in1=wk_bc,
                    op0=mybir.AluOpType.bypass, op1=mybir.AluOpType.mult,
                    accum_out=kcol[:, t:t + 1])
            nc.vector.tensor_scalar(out=qcol, in0=qcol, scalar1=bq_bc, scalar2=None,
                                    op0=mybir.AluOpType.add)
            nc.vector.tensor_scalar(out=kcol, in0=kcol, scalar1=bk_bc, scalar2=None,
                                    op0=mybir.AluOpType.add)

            # kcol [128, 16] -> kk_bcast [128, 2048] (channel-major order).
            kT_ps = ptr.tile([NT, 128], F32)
            nc.tensor.transpose(out=kT_ps, in_=kcol, identity=identity)
            kT_sb = small.tile([NT, 128], F32)
            nc.vector.tensor_copy(kT_sb, kT_ps)
            kk_row = small.tile([1, C], F32)
            nc.sync.dma_start(out=kk_row, in_=kT_sb)
            kk_bc = kkpool.tile([128, C], F32)
            nc.gpsimd.partition_broadcast(kk_bc, kk_row)

            imp_ps = pimp.tile([1, C], F32)
            for t in range(NT):
                e_t = epool.tile([128, C], F32, tag="et")
                nc.scalar.activation(out=e_t, in_=kk_bc,
                                     func=mybir.ActivationFunctionType.Exp,
                                     bias=0.0, scale=qcol[:, t:t + 1],
                                     accum_out=zcol[:, t:t + 1])
                nc.vector.reciprocal(rcol[:, t:t + 1], zcol[:, t:t + 1])
                for j in range(4):
                    nc.tensor.matmul(
                        out=imp_ps[:, j * 512:(j + 1) * 512],
                        lhsT=rcol[:, t:t + 1],
                        rhs=e_t[:, j * 512:(j + 1) * 512],
                        start=(t == 0), stop=(t == NT - 1))
            imp_sb = small.tile([1, C], F32)
            nc.vector.tensor_copy(imp_sb, imp_ps)
            nc.sync.dma_start(out=imp.ap()[b:b + 1], in_=imp_sb)
    nc.compile()
    return nc


M_PE = 1280                 # m in [0, M_PE) summed on PE (fp32 matmuls)
PE_CHUNKS = [(0, 512), (512, 512), (1024, 256)]
NDVE = (C - M_PE) // 128    # 6 transposed tiles summed on DVE


def build_imp_kernel_v2(repeat=1):
    """Split column-sum: PE handles m<[0,M_PE); DVE handles the rest via a
    second (transposed-orientation) exp pass with fused multiply+accum."""
    nc = bacc_mod.Bacc("TRN2", target_bir_lowering=False)
    x = nc.declare_dram_parameter("x", [BPC, C, D], F32, isOutput=False)
    wq = nc.declare_dram_parameter("wq", [D], F32, isOutput=False)
    wk = nc.declare_dram_parameter("wk", [D], F32, isOutput=False)
    bqk = nc.declare_dram_parameter("bqk", [2], F32, isOutput=False)
    imp = nc.declare_dram_parameter("imp", [BPC, C], F32, isOutput=True)

    with TileContext(nc) as tc, ExitStack() as ctx:
        const = ctx.enter_context(tc.tile_pool(name="const", bufs=1))
        xpool = ctx.enter_context(tc.tile_pool(name="xpool", bufs=2))
        epool = ctx.enter_context(tc.tile_pool(name="epool", bufs=4))
        bcpool = ctx.enter_context(tc.tile_pool(name="bcpool", bufs=2))
        qk = ctx.enter_context(tc.tile_pool(name="qk", bufs=2))
        small = ctx.enter_context(tc.tile_pool(name="small", bufs=2))
        big_scratch = ctx.enter_context(tc.tile_pool(name="bigs", bufs=1))
        pimp = ctx.enter_context(tc.tile_pool(name="pimp", bufs=2, space="PSUM"))
        ptr = ctx.enter_context(tc.tile_pool(name="ptr", bufs=2, space="PSUM"))

        def bcast_ap(handle, offset, n):
            a = handle.ap()
            return bass.AP(tensor=a.tensor, offset=offset, ap=[[0, 128], [1, n]])

        wq_bc = const.tile([128, D], F32)
        nc.gpsimd.dma_start(out=wq_bc, in_=bcast_ap(wq, 0, D))
        wk_bc = const.tile([128, D], F32)
        nc.gpsimd.dma_start(out=wk_bc, in_=bcast_ap(wk, 0, D))
        bq_bc = const.tile([128, 1], F32)
        nc.gpsimd.dma_start(out=bq_bc, in_=bcast_ap(bqk, 0, 1))
        bk_bc = const.tile([128, 1], F32)
        nc.gpsimd.dma_start(out=bk_bc, in_=bcast_ap(bqk, 1, 1))
        identity = const.tile([128, 128], F32)
        make_identity(nc, identity)

        dve_scratch = big_scratch.tile([128, C], F32)

        def col_to_row(col_ap, n, name):
            """[128, n] column layout -> [1, 128*n] row (channel-major) +
            broadcast to [128, 128*n]."""
            t_ps = ptr.tile([NT, 128], F32, name=f"tps_{name}", tag="tr")
            nc.tensor.transpose(out=t_ps[:n, :], in_=col_ap, identity=identity)
            t_sb = small.tile([NT, 128], F32, name=f"tsb_{name}", tag="tsb")
            nc.vector.tensor_copy(t_sb[:n, :], t_ps[:n, :])
            row = small.tile([1, C], F32, name=f"row_{name}", tag="row")
            nc.sync.dma_start(out=row[:, :n * 128], in_=t_sb[:n, :])
            bc = bcpool.tile([128, C], F32, name=f"bc_{name}", tag=f"bc_{name}")
            nc.gpsimd.partition_broadcast(bc[:, :n * 128], row[:, :n * 128])
            return row, bc

        for it in range(BPC * repeat):
            b = it % BPC
            x_t = xpool.tile([128, NT, D], F32)
            nc.sync.dma_start(out=x_t, in_=x.ap()[b].rearrange("(t p) d -> p t d", p=128))

            qcol = qk.tile([128, NT], F32)
            kcol = qk.tile([128, NT], F32)
            zcol = qk.tile([128, NT], F32)
            rcol = qk.tile([128, NT], F32)
            scratch = qk.tile([128, D], F32)
            for t in range(NT):
                nc.vector.scalar_tensor_tensor(
                    out=scratch, in0=x_t[:, t, :], scalar=0.0, in1=wk_bc,
                    op0=mybir.AluOpType.bypass, op1=mybir.AluOpType.mult,
                    accum_out=kcol[:, t:t + 1])
            nc.vector.tensor_scalar(out=kcol, in0=kcol, scalar1=bk_bc, scalar2=None,
                                    op0=mybir.AluOpType.add)
            for t in range(NT):
                nc.vector.scalar_tensor_tensor(
                    out=scratch, in0=x_t[:, t, :], scalar=0.0, in1=wq_bc,
                    op0=mybir.AluOpType.bypass, op1=mybir.AluOpType.mult,
                    accum_out=qcol[:, t:t + 1])
            nc.vector.tensor_scalar(out=qcol, in0=qcol, scalar1=bq_bc, scalar2=None,
                                    op0=mybir.AluOpType.add)

            _, kk_bc = col_to_row(kcol, NT, "k")
            _, q_bc = col_to_row(qcol, NT, "q")

            # Pass 1 (orientation c-partition): exp + Z-accum; PE partial sums
            # over m < M_PE.
            imp_ps = pimp.tile([1, M_PE], F32)
            for t in range(NT):
                e_t = epool.tile([128, C], F32, tag="et")
                nc.scalar.activation(out=e_t, in_=kk_bc,
                                     func=mybir.ActivationFunctionType.Exp,
                                     bias=0.0, scale=qcol[:, t:t + 1],
                                     accum_out=zcol[:, t:t + 1])
                nc.vector.reciprocal(rcol[:, t:t + 1], zcol[:, t:t + 1])
                for (off, width) in PE_CHUNKS:
                    nc.tensor.matmul(
                        out=imp_ps[:, off:off + width],
                        lhsT=rcol[:, t:t + 1],
                        rhs=e_t[:, off:off + width],
                        start=(t == 0), stop=(t == NT - 1))

            # r broadcast for the DVE part.
            _, r_bc = col_to_row(rcol, NT, "r")

            # Pass 2 (orientation m-partition) for m in [M_PE, C): one ACT exp
            # + one DVE fused multiply-accum per 128-m tile.
            impT = qk.tile([128, NDVE], F32)
            for td in range(NDVE):
                et_t = epool.tile([128, C], F32, tag="et")
                nc.scalar.activation(out=et_t, in_=q_bc,
                                     func=mybir.ActivationFunctionType.Exp,
                                     bias=0.0,
                                     scale=kcol[:, M_PE // 128 + td:M_PE // 128 + td + 1])
                nc.vector.scalar_tensor_tensor(
                    out=dve_scratch, in0=et_t, scalar=0.0, in1=r_bc,
                    op0=mybir.AluOpType.bypass, op1=mybir.AluOpType.mult,
                    accum_out=impT[:, td:td + 1])

            # Assemble: [0, M_PE) from PSUM; [M_PE, C) from transposed impT.
            impT_ps = ptr.tile([NT, 128], F32, name="impT_ps", tag="tr")
            nc.tensor.transpose(out=impT_ps[:NDVE, :], in_=impT, identity=identity)
            impT_sb = small.tile([NT, 128], F32, name="impT_sb", tag="tsb")
            nc.vector.tensor_copy(impT_sb[:NDVE, :], impT_ps[:NDVE, :])
            imp_sb = small.tile([1, M_PE], F32, name="imp_sb", tag="imp_sb")
            nc.vector.tensor_copy(imp_sb, imp_ps)
            nc.sync.dma_start(out=imp.ap()[b:b + 1, 0:M_PE], in_=imp_sb)
            nc.sync.dma_start(
                out=imp.ap()[b:b + 1, M_PE:C].rearrange("o (g p) -> (o g) p", p=128),
                in_=impT_sb[:NDVE, :])
    nc.compile()
    return nc


def build_imp_kernel_v3(repeat=1, ndve=NDVE):
    """Software-pipelined: per iteration, emit (a) x-load + k/q projections +
    broadcasts for batch b+1, (b) transposed-pass colsums for batch b-1,
    (c) main exp/colsum pass for batch b — so ACT/DVE/PE queues never drain
    at batch boundaries."""
    m_pe = C - 128 * ndve
    chunks = []
    off = 0
    while off < m_pe:
        w = min(512, m_pe - off)
        chunks.append((off, w))
        off += w
    nc = bacc_mod.Bacc("TRN2", target_bir_lowering=False)
    x = nc.declare_dram_parameter("x", [BPC, C, D], F32, isOutput=False)
    wq = nc.declare_dram_parameter("wq", [D], F32, isOutput=False)
    wk = nc.declare_dram_parameter("wk", [D], F32, isOutput=False)
    bqk = nc.declare_dram_parameter("bqk", [2], F32, isOutput=False)
    imp = nc.declare_dram_parameter("imp", [BPC, C], F32, isOutput=True)

    NB = BPC * repeat

    with TileContext(nc) as tc, ExitStack() as ctx:
        const = ctx.enter_context(tc.tile_pool(name="const", bufs=1))
        xpool = ctx.enter_context(tc.tile_pool(name="xpool", bufs=2))
        epool = ctx.enter_context(tc.tile_pool(name="epool", bufs=5))
        bcpool = ctx.enter_context(tc.tile_pool(name="bcpool", bufs=2))
        qk = ctx.enter_context(tc.tile_pool(name="qk", bufs=2))
        small = ctx.enter_context(tc.tile_pool(name="small", bufs=2))
        big_scratch = ctx.enter_context(tc.tile_pool(name="bigs", bufs=1))
        pimp = ctx.enter_context(tc.tile_pool(name="pimp", bufs=2, space="PSUM"))
        ptr = ctx.enter_context(tc.tile_pool(name="ptr", bufs=2, space="PSUM"))

        def bcast_ap(handle, offset, n):
            a = handle.ap()
            return bass.AP(tensor=a.tensor, offset=offset, ap=[[0, 128], [1, n]])

        wq_bc = const.tile([128, D], F32)
        nc.gpsimd.dma_start(out=wq_bc, in_=bcast_ap(wq, 0, D))
        wk_bc = const.tile([128, D], F32)
        nc.gpsimd.dma_start(out=wk_bc, in_=bcast_ap(wk, 0, D))
        bq_bc = const.tile([128, 1], F32)
        nc.gpsimd.dma_start(out=bq_bc, in_=bcast_ap(bqk, 0, 1))
        bk_bc = const.tile([128, 1], F32)
        nc.gpsimd.dma_start(out=bk_bc, in_=bcast_ap(bqk, 1, 1))
        identity = const.tile([128, 128], F32)
        make_identity(nc, identity)

        dve_scratch = big_scratch.tile([128, C], F32)

        def col_to_row(col_ap, name, w_bc=None):
            t_ps = ptr.tile([NT, 128], F32, name=f"tps_{name}", tag="tr")
            nc.tensor.transpose(out=t_ps, in_=col_ap, identity=identity)
            t_sb = small.tile([NT, 128], F32, name=f"tsb_{name}", tag="tsb")
            nc.vector.tensor_copy(t_sb, t_ps)
            row = small.tile([1, C], F32, name=f"row_{name}", tag="row")
            nc.sync.dma_start(out=row, in_=t_sb)
            bc = bcpool.tile([128, C], F32, name=f"bc_{name}", tag=f"bc_{name}")
            nc.gpsimd.partition_broadcast(bc, row)
            return bc

        st = {}  # per-batch live state

        def stage_load_proj(nb):
            x_t = xpool.tile([128, NT, D], F32)
            half = NT // 2
            src = x.ap()[nb % BPC].rearrange("(t p) d -> p t d", p=128)
            nc.sync.dma_start(out=x_t[:, :half, :], in_=src[:, :half, :])
            nc.sync.dma_start(out=x_t[:, half:, :], in_=src[:, half:, :])
            qcol = qk.tile([128, NT], F32)
            kcol = qk.tile([128, NT], F32)
            scratch = qk.tile([128, D], F32)
            for t in range(NT):
                nc.vector.scalar_tensor_tensor(
                    out=scratch, in0=x_t[:, t, :], scalar=0.0, in1=wk_bc,
                    op0=mybir.AluOpType.bypass, op1=mybir.AluOpType.mult,
                    accum_out=kcol[:, t:t + 1])
            nc.vector.tensor_scalar(out=kcol, in0=kcol, scalar1=bk_bc, scalar2=None,
                                    op0=mybir.AluOpType.add)
            kk_bc = col_to_row(kcol, "k")
            for t in range(NT):
                nc.vector.scalar_tensor_tensor(
                    out=scratch, in0=x_t[:, t, :], scalar=0.0, in1=wq_bc,
                    op0=mybir.AluOpType.bypass, op1=mybir.AluOpType.mult,
                    accum_out=qcol[:, t:t + 1])
            nc.vector.tensor_scalar(out=qcol, in0=qcol, scalar1=bq_bc, scalar2=None,
                                    op0=mybir.AluOpType.add)
            q_bc = col_to_row(qcol, "q")
            st[nb] = dict(qcol=qcol, kcol=kcol, kk_bc=kk_bc, q_bc=q_bc)

        def stage_pass2(nb):
            s = st[nb]
            impT = qk.tile([128, max(ndve, 1)], F32)
            for td in range(ndve):
                et_t = epool.tile([128, C], F32, tag="et")
                nc.scalar.activation(out=et_t, in_=s["q_bc"],
                                     func=mybir.ActivationFunctionType.Exp,
                                     bias=0.0,
                                     scale=s["kcol"][:, m_pe // 128 + td:m_pe // 128 + td + 1])
                nc.vector.scalar_tensor_tensor(
                    out=dve_scratch, in0=et_t, scalar=0.0, in1=s["r_bc"],
                    op0=mybir.AluOpType.bypass, op1=mybir.AluOpType.mult,
                    accum_out=impT[:, td:td + 1])
            s["impT"] = impT

        def stage_pass1(nb):
            s = st[nb]
            zcol = qk.tile([128, NT], F32)
            rcol = qk.tile([128, NT], F32)
            imp_ps = pimp.tile([1, m_pe], F32)
            for t in range(NT):
                e_t = epool.tile([128, C], F32, tag="et")
                nc.scalar.activation(out=e_t, in_=s["kk_bc"],
                                     func=mybir.ActivationFunctionType.Exp,
                                     bias=0.0, scale=s["qcol"][:, t:t + 1],
                                     accum_out=zcol[:, t:t + 1])
                nc.vector.reciprocal(rcol[:, t:t + 1], zcol[:, t:t + 1])
                for (coff, w) in chunks:
                    nc.tensor.matmul(
                        out=imp_ps[:, coff:coff + w],
                        lhsT=rcol[:, t:t + 1],
                        rhs=e_t[:, coff:coff + w],
                        start=(t == 0), stop=(t == NT - 1))
            s["r_bc"] = col_to_row(rcol, "r")
            s["imp_ps"] = imp_ps

        def stage_finalize(nb):
            s = st.pop(nb)
            b = nb % BPC
            impT_ps = ptr.tile([NT, 128], F32, name="impT_ps", tag="tr")
            nc.tensor.transpose(out=impT_ps[:ndve, :], in_=s["impT"][:, :ndve],
                                identity=identity)
            impT_sb = small.tile([NT, 128], F32, name="impT_sb", tag="tsb")
            nc.vector.tensor_copy(impT_sb[:ndve, :], impT_ps[:ndve, :])
            imp_sb = small.tile([1, m_pe], F32, name="imp_sb", tag="imp_sb")
            nc.vector.tensor_copy(imp_sb, s["imp_ps"])
            nc.sync.dma_start(out=imp.ap()[b:b + 1, 0:m_pe], in_=imp_sb)
            nc.sync.dma_start(
                out=imp.ap()[b:b + 1, m_pe:C].rearrange("o (g p) -> (o g) p", p=128),
                in_=impT_sb[:ndve, :])

        def stage_pass2_tile(nb, td):
            s = st[nb]
            if td == 0:
                s["impT"] = qk.tile([128, max(ndve, 1)], F32, name="impT", tag="impT")
            et_t = epool.tile([128, C], F32, tag="et")
            nc.scalar.activation(out=et_t, in_=s["q_bc"],
                                 func=mybir.ActivationFunctionType.Exp,
                                 bias=0.0,
                                 scale=s["kcol"][:, m_pe // 128 + td:m_pe // 128 + td + 1])
            nc.vector.scalar_tensor_tensor(
                out=dve_scratch, in0=et_t, scalar=0.0, in1=s["r_bc"],
                op0=mybir.AluOpType.bypass, op1=mybir.AluOpType.mult,
                accum_out=s["impT"][:, td:td + 1])

        def stage_pass1_interleaved(nb):
            """pass1(nb) with pass2(nb-1) tiles woven into the tail so the
            ACT queue never drains at the batch boundary."""
            s = st[nb]
            zcol = qk.tile([128, NT], F32)
            rcol = qk.tile([128, NT], F32)
            imp_ps = pimp.tile([1, m_pe], F32)
            start_iv = NT - ndve
            for t in range(NT):
                if nb >= 1 and t >= start_iv:
                    stage_pass2_tile(nb - 1, t - start_iv)
                e_t = epool.tile([128, C], F32, tag="et")
                nc.scalar.activation(out=e_t, in_=s["kk_bc"],
                                     func=mybir.ActivationFunctionType.Exp,
                                     bias=0.0, scale=s["qcol"][:, t:t + 1],
                                     accum_out=zcol[:, t:t + 1])
                nc.vector.reciprocal(rcol[:, t:t + 1], zcol[:, t:t + 1])
                for (coff, w) in chunks:
                    nc.tensor.matmul(
                        out=imp_ps[:, coff:coff + w],
                        lhsT=rcol[:, t:t + 1],
                        rhs=e_t[:, coff:coff + w],
                        start=(t == 0), stop=(t == NT - 1))
            s["r_bc"] = col_to_row(rcol, "r")
            s["imp_ps"] = imp_ps

        stage_load_proj(0)
        for nb in range(NB):
            if nb + 1 < NB:
                stage_load_proj(nb + 1)
            stage_pass1_interleaved(nb)
            if nb >= 1:
                stage_finalize(nb - 1)
        for td in range(ndve):
            stage_pass2_tile(NB - 1, td)
        stage_finalize(NB - 1)
    nc.compile()
    return nc


def build_gather_kernel(repeat=1):
    nc = bacc_mod.Bacc("TRN2", target_bir_lowering=False)
    x = nc.declare_dram_parameter("x", [BPC, C, D], F32, isOutput=False)
    idx = nc.declare_dram_parameter("idx", [BPC, 128, KPAD // 16], I16, isOutput=False)
    sparse = nc.declare_dram_parameter("sparse", [BPC, KPAD, D], F32, isOutput=True)

    with TileContext(nc) as tc, ExitStack() as ctx:
        pool = ctx.enter_context(tc.tile_pool(name="g", bufs=2))
        for b in range(BPC * repeat):
            b = b % BPC
            idx_sb = pool.tile([128, KPAD // 16], I16)
            nc.sync.dma_start(out=idx_sb, in_=idx.ap()[b])
            g_sb = pool.tile([128, GD, D], F32)
            nc.gpsimd.dma_gather(
                out_ap=g_sb, in_ap=x.ap()[b], idxs_ap=idx_sb,
                num_idxs=KPAD, num_idxs_reg=KPAD, elem_size=D)
            nc.sync.dma_start(
                out=sparse.ap()[b].rearrange("(g p) d -> p g d", p=128),
                in_=g_sb)
    nc.compile()
    return nc


class PjrtRunner:
    """Cached jitted PJRT executor for one Bass module (mirrors
    bass2jax.run_bass_via_pjrt multi-core path, but reusable so repeat
    calls don't re-trace, enabling steady-state timing)."""

    def __init__(self, nc, n_cores=NCORES):
        import jax
        from jax.sharding import Mesh, PartitionSpec
        try:
            from jax.experimental.shard_map import shard_map
        except ImportError:  # newer jax
            from jax.shard_map import shard_map
        from concourse import bass2jax as b2j

        b2j.install_neuronx_cc_hook()
        self.jax = jax
        self.nc = nc
        self.n_cores = n_cores
        partition_name = (nc.partition_id_tensor.name
                          if nc.partition_id_tensor else None)
        in_names, out_names, out_avals, zero_outs = [], [], [], []
        for alloc in nc.m.functions[0].allocations:
            if not isinstance(alloc, mybir.MemoryLocationSet):
                continue
            name = alloc.memorylocations[0].name
            if alloc.kind == "ExternalInput":
                if name != partition_name and name != (
                        nc.dbg_addr.name if nc.dbg_addr else None):
                    in_names.append(name)
            elif alloc.kind == "ExternalOutput":
                shape = tuple(alloc.tensor_shape)
                dtype = mybir.dt.np(alloc.dtype)
                out_names.append(name)
                out_avals.append(jax.core.ShapedArray(shape, dtype))
                zero_outs.append(np.zeros(shape, dtype))
        self.in_names = list(in_names)
        self.out_names = out_names
        self.out_avals = out_avals
        self.zero_outs = zero_outs
        n_params = len(in_names)
        n_outs = len(out_names)
        all_in_names = list(in_names) + list(out_names)
        dbg_name = nc.dbg_addr.name if nc.dbg_addr else None
        if dbg_name is not None:
            all_in_names.append(dbg_name)
        if partition_name is not None:
            all_in_names.append(partition_name)
        self._dbg_name = dbg_name

        def _body(*args):
            operands = list(args)
            if dbg_name is not None:
                operands.append(jax.numpy.zeros((1, 2), np.uint32))
            if partition_name is not None:
                operands.append(b2j.partition_id_tensor())
            outs = b2j._bass_exec_p.bind(
                *operands,
                out_avals=tuple(out_avals),
                in_names=tuple(all_in_names),
                out_names=tuple(out_names),
                lowering_input_output_aliases=(),
                sim_require_finite=True,
                sim_require_nnan=True,
                nc=nc,
            )
            return tuple(outs)

        devices = jax.devices()[:n_cores]
        self.mesh = Mesh(np.asarray(devices), ("core",))
        in_specs = (PartitionSpec("core"),) * (n_params + n_outs)
        out_specs = (PartitionSpec("core"),) * n_outs
        self.fn = jax.jit(
            shard_map(_body, mesh=self.mesh, in_specs=in_specs,
                      out_specs=out_specs, check_rep=False),
            keep_unused=True,
        )

    def _concat_inputs(self, in_maps):
        return [np.concatenate([np.asarray(in_maps[c][n])
                                for c in range(self.n_cores)], axis=0)
                for n in self.in_names]

    def __call__(self, in_maps):
        concat_in = self._concat_inputs(in_maps)
        concat_zeros = [np.zeros((self.n_cores * z.shape[0], *z.shape[1:]),
                                 z.dtype) for z in self.zero_outs]
        outs = self.fn(*concat_in, *concat_zeros)
        return [
            {n: np.asarray(outs[i]).reshape(self.n_cores, *self.out_avals[i].shape)[c]
             for i, n in enumerate(self.out_names)}
            for c in range(self.n_cores)
        ]

    def device_args(self, in_maps):
        import jax
        from jax.sharding import NamedSharding, PartitionSpec
        sh = NamedSharding(self.mesh, PartitionSpec("core"))
        concat_in = self._concat_inputs(in_maps)
        concat_zeros = [np.zeros((self.n_cores * z.shape[0], *z.shape[1:]),
                                 z.dtype) for z in self.zero_outs]
        return [jax.device_put(a, sh) for a in concat_in + concat_zeros]

    def timed_ns(self, in_maps, iters=20, warmup=3):
        import time
        args = self.device_args(in_maps)
        for _ in range(warmup):
            outs = self.fn(*args)
            self.jax.block_until_ready(outs)
        ts = []
        for _ in range(iters):
            t0 = time.perf_counter()
            outs = self.fn(*args)
            self.jax.block_until_ready(outs)
            ts.append(time.perf_counter() - t0)
        return int(min(ts) * 1e9)

    def _chained_fn(self, m):
        """jit fn executing the bass program m times serially (call i's
        outputs feed call i+1's output-seed operands: data dep, no CSE)."""
        import jax
        from jax.sharding import PartitionSpec
        try:
            from jax.experimental.shard_map import shard_map
        except ImportError:
            from jax.shard_map import shard_map
        from concourse import bass2jax as b2j
        nc = self.nc
        n_params = len(self.in_names)
        n_outs = len(self.out_names)
        partition_name = (nc.partition_id_tensor.name
                          if nc.partition_id_tensor else None)
        all_in_names = list(self.in_names) + list(self.out_names)
        if self._dbg_name is not None:
            all_in_names.append(self._dbg_name)
        if partition_name is not None:
            all_in_names.append(partition_name)
        out_avals = tuple(self.out_avals)
        out_names = tuple(self.out_names)
        dbg_name = self._dbg_name

        def _body(*args):
            ins = list(args[:n_params])
            zouts = list(args[n_params:])
            for _ in range(m):
                operands = ins + zouts
                if dbg_name is not None:
                    operands.append(jax.numpy.zeros((1, 2), np.uint32))
                if partition_name is not None:
                    operands.append(b2j.partition_id_tensor())
                outs = b2j._bass_exec_p.bind(
                    *operands,
                    out_avals=out_avals,
                    in_names=tuple(all_in_names),
                    out_names=out_names,
                    lowering_input_output_aliases=(),
                    sim_require_finite=True,
                    sim_require_nnan=True,
                    nc=nc,
                )
                zouts = list(outs)
            return tuple(outs)

        in_specs = (PartitionSpec("core"),) * (n_params + n_outs)
        out_specs = (PartitionSpec("core"),) * n_outs
        return jax.jit(
            shard_map(_body, mesh=self.mesh, in_specs=in_specs,
                      out_specs=out_specs, check_rep=False),
            keep_unused=True,
        )

    def chained_timed_ns(self, in_maps, chain=8, iters=8, warmup=2):
        """Per-execution ns via (T(chain) - T(1)) / (chain - 1)."""
        import time
        args = self.device_args(in_maps)
        f1 = self._chained_fn(1)
        fm = self._chained_fn(chain)

        def best(fn):
            for _ in range(warmup):
                self.jax.block_until_ready(fn(*args))
            ts = []
            for _ in range(iters):
                t0 = time.perf_counter()
                self.jax.block_until_ready(fn(*args))
                ts.append(time.perf_counter() - t0)
            return min(ts)

        t1, tm = best(f1), best(fm)
        return int((tm - t1) / (chain - 1) * 1e9)


_CACHE = {}


_BUILDERS = {"imp": build_imp_kernel_v3, "gather": build_gather_kernel}


def _runner(key, repeat=1):
    ckey = f"{key}_r{repeat}"
    if ckey not in _CACHE:
        _CACHE[ckey] = PjrtRunner(_BUILDERS[key](repeat=repeat))
    return _CACHE[ckey]


def _in_maps_imp(x, Wq, Wk, bqk):
    return [{"x": x[c * BPC:(c + 1) * BPC], "wq": Wq, "wk": Wk, "bqk": bqk}
            for c in range(NCORES)]


def _in_maps_gather(x, idx16):
    return [{"x": x[c * BPC:(c + 1) * BPC], "idx": idx16[c * BPC:(c + 1) * BPC]}
            for c in range(NCORES)]


def _pack_idx16(topk_idx):
    # Flat rank order i lives at [i % 16, i // 16]; the 16-partition block is
    # replicated to all 8 Q7-core partition groups (each core reads its own).
    flat = np.zeros((B, KPAD), dtype=np.int16)
    flat[:, :K] = topk_idx
    blk = flat.reshape(B, KPAD // 16, 16).transpose(0, 2, 1)  # [B, 16, 40]
    return np.tile(blk, (1, 8, 1))  # [B, 128, 40]


def _median_call_ms(runner, in_maps, iters=12):
    import time
    import jax
    args = runner.device_args(in_maps)
    jax.block_until_ready(runner.fn(*args))
    ts = []
    for _ in range(iters):
        t0 = time.perf_counter()
        jax.block_until_ready(runner.fn(*args))
        ts.append((time.perf_counter() - t0) * 1e3)
    return float(np.median(ts))


def timed_hw_ns(inputs, rep=17, iters=12):
    """Per-execution device time of both launches via the repeat-variant
    differential: build each kernel at repeat=1 and repeat=rep (same I/O,
    rep x the work), time both with device-resident inputs, slope =
    (T(rep)-T(1))/(rep-1). Removes transfer/dispatch overhead, which
    dominates per-call wall time under the axon relay."""
    x = np.ascontiguousarray(np.asarray(inputs["x"], dtype=np.float32))
    Wq = np.asarray(inputs["Wq"], dtype=np.float32).reshape(-1)
    Wk = np.asarray(inputs["Wk"], dtype=np.float32).reshape(-1)
    bqk = np.array([np.asarray(inputs["bq"]).reshape(-1)[0],
                    np.asarray(inputs["bk"]).reshape(-1)[0]], dtype=np.float32)
    im = _in_maps_imp(x, Wq, Wk, bqk)
    t1 = _median_call_ms(_runner("imp", 1), im, iters)
    tr = _median_call_ms(_runner("imp", rep), im, iters)
    imp_ns = max(0.0, (tr - t1) / (rep - 1)) * 1e6
    res1 = _runner("imp", 1)(im)
    imp = np.concatenate([res1[c]["imp"] for c in range(NCORES)], axis=0)
    topk_idx = np.argsort(-imp, axis=-1, kind="stable")[:, :K].astype(np.int32)
    gm = _in_maps_gather(x, _pack_idx16(topk_idx))
    grep = max(rep, 65)  # gather is short; needs a larger multiple for signal
    g1 = _median_call_ms(_runner("gather", 1), gm, iters)
    gr = _median_call_ms(_runner("gather", grep), gm, iters)
    g_ns = max(0.0, (gr - g1) / (grep - 1)) * 1e6
    print(f"  launch1 (imp): {imp_ns:.0f} ns, launch2 (gather): {g_ns:.0f} ns")
    return int(imp_ns + g_ns)


def kernel(**inputs):
    x = np.ascontiguousarray(np.asarray(inputs["x"], dtype=np.float32))
    Wq = np.asarray(inputs["Wq"], dtype=np.float32).reshape(-1)
    Wk = np.asarray(inputs["Wk"], dtype=np.float32).reshape(-1)
    bq = np.asarray(inputs["bq"], dtype=np.float32).reshape(-1)
    bk = np.asarray(inputs["bk"], dtype=np.float32).reshape(-1)
    assert x.shape == (B, C, D)

    core_ids = list(range(NCORES))
    bqk = np.array([bq[0], bk[0]], dtype=np.float32)
    res1 = _runner("imp")(_in_maps_imp(x, Wq, Wk, bqk))
    imp = np.concatenate([res1[c]["imp"] for c in core_ids], axis=0)

    # Top-k on host: stable argsort desc == jax.lax.top_k ordering.
    topk_idx = np.argsort(-imp, axis=-1, kind="stable")[:, :K].astype(np.int32)

    # Pack indices for dma_gather: flat rank order i lives at [i % 16, i // 16]
    # of the first 16 partitions; pad ranks K..KPAD with 0 (extra rows are
    # gathered but sliced off on the host).
    idx16 = _pack_idx16(topk_idx)
    res2 = _runner("gather")(_in_maps_gather(x, idx16))
    sparse = np.concatenate(
        [res2[c]["sparse"][:, :K, :] for c in core_ids], axis=0)

    return sparse, topk_idx, K


# revision 20
# speedup vs baseline: 1.0977x; 1.0977x over previous
"""Trainium2 Bass kernel for nn_ChannelWiseAttention (topk_masking).

Math (KDIM=1 makes attention scores a rank-1 outer product):
  q = x @ Wq + bq, kk = x @ Wk + bk               [B, C]
  S[b,c,m] = q[b,c]*kk[b,m]; W = softmax(S, -1); imp = mean_c W   [B, C]
  topk(imp) -> indices (sorted desc), gather those rows of x.

Plan (pure data parallel: B=32 -> 4 batches per core on 8 cores):
  Launch 1 (imp, software-pipelined across batches - build_imp_kernel_v3):
    - x[b] DMA'd as [128, 16, 512] (channel c = t*128 + p).
    - DVE: q/kk via fused scalar_tensor_tensor multiply+accum against
      partition-broadcast Wq/Wk rows.
    - PE-transpose + SBUF DMA + GpSimd partition_broadcast build [128, 2048]
      broadcasts of kk, q, and r = 1/Z.
    - Pass 1 (partition = c): ACT fuses outer-product+exp+row-sum per
      128-channel tile: E_t = exp(kk_bc * q_p), accum_out -> Z. (No row-max
      subtraction: |S| <= ~25 stays comfortably in fp32 exp range.) PE fp32
      matmuls (lhsT = 1/Z column) accumulate imp[m] for m < M_PE=1280.
    - Pass 2 (partition = m, for m >= M_PE): ACT re-computes the transposed
      exp tiles; DVE does the weighted column-sum via fused multiply+accum
      with the r broadcast. Splitting the column-sum PE/DVE balances engines
      (fp32 moving operand runs at 4 cyc/row on the PE).
    - Pass 2 of batch b-1 is interleaved into the tail of pass 1 of batch b,
      and batch b+1's projections are emitted first, so ACT/PE/DVE queues
      stay fed across batch boundaries.
  Host: stable argsort of imp == jax.lax.top_k ordering (ties -> lower index).
  Launch 2: GpSimd dma_gather of the selected 614 rows (padded to 640) of x
    from HBM, written to [B, 640, 512]; host slices to 614.

Accuracy note: reference top-k ordering has adjacent near-ties at ~1e-9
relative; any independent f32 (even f64) evaluation swaps a few of them.
Measured sparse_feat Frobenius rel err ~= 3% equals that floor (selection
set matches exactly; only intra-top-k order of near-ties differs).
"""

import numpy as np
from contextlib import ExitStack

import concourse.bass as bass
import concourse.bacc as bacc_mod
import concourse.mybir as mybir
from concourse.tile import TileContext
from concourse.bass_utils import run_bass_kernel_spmd
from concourse.masks import make_identity

F32 = mybir.dt.float32
I16 = mybir.dt.int16

B, C, D = 32, 2048, 512
NCORES = 8
BPC = B // NCORES          # batches per core
NT = C // 128              # 16 channel tiles per batch
K = max(1, int(C * 0.3))   # 614
KPAD = 640                 # next multiple of 128
GD = KPAD // 128           # 5


def build_imp_kernel(repeat=1):
    nc = bacc_mod.Bacc("TRN2", target_bir_lowering=False)
    x = nc.declare_dram_parameter("x", [BPC, C, D], F32, isOutput=False)
    wq = nc.declare_dram_parameter("wq", [D], F32, isOutput=False)
    wk = nc.declare_dram_parameter("wk", [D], F32, isOutput=False)
    bqk = nc.declare_dram_parameter("bqk", [2], F32, isOutput=False)
    imp = nc.declare_dram_parameter("imp", [BPC, C], F32, isOutput=True)

    with TileContext(nc) as tc, ExitStack() as ctx:
        const = ctx.enter_context(tc.tile_pool(name="const", bufs=1))
        xpool = ctx.enter_context(tc.tile_pool(name="xpool", bufs=2))
        epool = ctx.enter_context(tc.tile_pool(name="epool", bufs=3))
        kkpool = ctx.enter_context(tc.tile_pool(name="kkpool", bufs=2))
        qk = ctx.enter_context(tc.tile_pool(name="qk", bufs=2))
        small = ctx.enter_context(tc.tile_pool(name="small", bufs=2))
        pimp = ctx.enter_context(tc.tile_pool(name="pimp", bufs=1, space="PSUM"))
        ptr = ctx.enter_context(tc.tile_pool(name="ptr", bufs=2, space="PSUM"))

        # Broadcast W rows across all partitions (DMA partition-stride 0).
        def bcast_ap(handle, offset, n):
            a = handle.ap()
            return bass.AP(tensor=a.tensor, offset=offset, ap=[[0, 128], [1, n]])

        wq_bc = const.tile([128, D], F32)
        nc.gpsimd.dma_start(out=wq_bc, in_=bcast_ap(wq, 0, D))
        wk_bc = const.tile([128, D], F32)
        nc.gpsimd.dma_start(out=wk_bc, in_=bcast_ap(wk, 0, D))
        bq_bc = const.tile([128, 1], F32)
        nc.gpsimd.dma_start(out=bq_bc, in_=bcast_ap(bqk, 0, 1))
        bk_bc = const.tile([128, 1], F32)
        nc.gpsimd.dma_start(out=bk_bc, in_=bcast_ap(bqk, 1, 1))
        identity = const.tile([128, 128], F32)
        make_identity(nc, identity)

        for b in range(BPC * repeat):
            b = b % BPC
            x_t = xpool.tile([128, NT, D], F32)
            nc.sync.dma_start(out=x_t, in_=x.ap()[b].rearrange("(t p) d -> p t d", p=128))

            qcol = qk.tile([128, NT], F32)
            kcol = qk.tile([128, NT], F32)
            zcol = qk.tile([128, NT], F32)
            rcol = qk.tile([128, NT], F32)
            scratch = qk.tile([128, D], F32)
            for t in range(NT):
                nc.vector.scalar_tensor_tensor(
                    out=scratch, in0=x_t[:, t, :], scalar=0.0, in1=wq_bc,
                    op0=mybir.AluOpType.bypass, op1=mybir.AluOpType.mult,
                    accum_out=qcol[:, t:t + 1])
                nc.vector.scalar_tensor_tensor(
                    out=scratch, in0=x_t[:, t, :], scalar=0.0, in1=wk_bc,
                    op0=mybir.AluOpType.bypass, op1=mybir.AluOpType.mult,
                    accum_out=kcol[:, t:t + 1])
            nc.vector.tensor_scalar(out=qcol, in0=qcol, scalar1=bq_bc, scalar2=None,
                                    op0=mybir.AluOpType.add)
            nc.vector.tensor_scalar(out=kcol, in0=kcol, scalar1=bk_bc, scalar2=None,
                                    op0=mybir.AluOpType.add)

            # kcol [128, 16] -> kk_bcast [128, 2048] (channel-major order).
            kT_ps = ptr.tile([NT, 128], F32)
            nc.tensor.transpose(out=kT_ps, in_=kcol, identity=identity)
            kT_sb = small.tile([NT, 128], F32)
            nc.vector.tensor_copy(kT_sb, kT_ps)
            kk_row = small.tile([1, C], F32)
            nc.sync.dma_start(out=kk_row, in_=kT_sb)
            kk_bc = kkpool.tile([128, C], F32)
            nc.gpsimd.partition_broadcast(kk_bc, kk_row)

            imp_ps = pimp.tile([1, C], F32)
            for t in range(NT):
                e_t = epool.tile([128, C], F32, tag="et")
                nc.scalar.activation(out=e_t, in_=kk_bc,
                                     func=mybir.ActivationFunctionType.Exp,
                                     bias=0.0, scale=qcol[:, t:t + 1],
                                     accum_out=zcol[:, t:t + 1])
                nc.vector.reciprocal(rcol[:, t:t + 1], zcol[:, t:t + 1])
                for j in range(4):
                    nc.tensor.matmul(
                        out=imp_ps[:, j * 512:(j + 1) * 512],
                        lhsT=rcol[:, t:t + 1],
                        rhs=e_t[:, j * 512:(j + 1) * 512],
                        start=(t == 0), stop=(t == NT - 1))
            imp_sb = small.tile([1, C], F32)
            nc.vector.tensor_copy(imp_sb, imp_ps)
            nc.sync.dma_start(out=imp.ap()[b:b + 1], in_=imp_sb)
    nc.compile()
    return nc


M_PE = 1280                 # m in [0, M_PE) summed on PE (fp32 matmuls)
PE_CHUNKS = [(0, 512), (512, 512), (1024, 256)]
NDVE = (C - M_PE) // 128    # 6 transposed tiles summed on DVE


def build_imp_kernel_v2(repeat=1):
    """Split column-sum: PE handles m<[0,M_PE); DVE handles the rest via a
    second (transposed-orientation) exp pass with fused multiply+accum."""
    nc = bacc_mod.Bacc("TRN2", target_bir_lowering=False)
    x = nc.declare_dram_parameter("x", [BPC, C, D], F32, isOutput=False)
    wq = nc.declare_dram_parameter("wq", [D], F32, isOutput=False)
    wk = nc.declare_dram_parameter("wk", [D], F32, isOutput=False)
    bqk = nc.declare_dram_parameter("bqk", [2], F32, isOutput=False)
    imp = nc.declare_dram_parameter("imp", [BPC, C], F32, isOutput=True)

    with TileContext(nc) as tc, ExitStack() as ctx:
        const = ctx.enter_context(tc.tile_pool(name="const", bufs=1))
        xpool = ctx.enter_context(tc.tile_pool(name="xpool", bufs=2))
        epool = ctx.enter_context(tc.tile_pool(name="epool", bufs=4))
        bcpool = ctx.enter_context(tc.tile_pool(name="bcpool", bufs=2))
        qk = ctx.enter_context(tc.tile_pool(name="qk", bufs=2))
        small = ctx.enter_context(tc.tile_pool(name="small", bufs=2))
        big_scratch = ctx.enter_context(tc.tile_pool(name="bigs", bufs=1))
        pimp = ctx.enter_context(tc.tile_pool(name="pimp", bufs=2, space="PSUM"))
        ptr = ctx.enter_context(tc.tile_pool(name="ptr", bufs=2, space="PSUM"))

        def bcast_ap(handle, offset, n):
            a = handle.ap()
            return bass.AP(tensor=a.tensor, offset=offset, ap=[[0, 128], [1, n]])

        wq_bc = const.tile([128, D], F32)
        nc.gpsimd.dma_start(out=wq_bc, in_=bcast_ap(wq, 0, D))
        wk_bc = const.tile([128, D], F32)
        nc.gpsimd.dma_start(out=wk_bc, in_=bcast_ap(wk, 0, D))
        bq_bc = const.tile([128, 1], F32)
        nc.gpsimd.dma_start(out=bq_bc, in_=bcast_ap(bqk, 0, 1))
        bk_bc = const.tile([128, 1], F32)
        nc.gpsimd.dma_start(out=bk_bc, in_=bcast_ap(bqk, 1, 1))
        identity = const.tile([128, 128], F32)
        make_identity(nc, identity)

        dve_scratch = big_scratch.tile([128, C], F32)

        def col_to_row(col_ap, n, name):
            """[128, n] column layout -> [1, 128*n] row (channel-major) +
            broadcast to [128, 128*n]."""
            t_ps = ptr.tile([NT, 128], F32, name=f"tps_{name}", tag="tr")
            nc.tensor.transpose(out=t_ps[:n, :], in_=col_ap, identity=identity)
            t_sb = small.tile([NT, 128], F32, name=f"tsb_{name}", tag="tsb")
            nc.vector.tensor_copy(t_sb[:n, :], t_ps[:n, :])
            row = small.tile([1, C], F32, name=f"row_{name}", tag="row")
            nc.sync.dma_start(out=row[:, :n * 128], in_=t_sb[:n, :])
            bc = bcpool.tile([128, C], F32, name=f"bc_{name}", tag=f"bc_{name}")
            nc.gpsimd.partition_broadcast(bc[:, :n * 128], row[:, :n * 128])
            return row, bc

        for it in range(BPC * repeat):
            b = it % BPC
            x_t = xpool.tile([128, NT, D], F32)
            nc.sync.dma_start(out=x_t, in_=x.ap()[b].rearrange("(t p) d -> p t d", p=128))

            qcol = qk.tile([128, NT], F32)
            kcol = qk.tile([128, NT], F32)
            zcol = qk.tile([128, NT], F32)
            rcol = qk.tile([128, NT], F32)
            scratch = qk.tile([128, D], F32)
            for t in range(NT):
                nc.vector.scalar_tensor_tensor(
                    out=scratch, in0=x_t[:, t, :], scalar=0.0, in1=wk_bc,
                    op0=mybir.AluOpType.bypass, op1=mybir.AluOpType.mult,
                    accum_out=kcol[:, t:t + 1])
            nc.vector.tensor_scalar(out=kcol, in0=kcol, scalar1=bk_bc, scalar2=None,
                                    op0=mybir.AluOpType.add)
            for t in range(NT):
                nc.vector.scalar_tensor_tensor(
                    out=scratch, in0=x_t[:, t, :], scalar=0.0, in1=wq_bc,
                    op0=mybir.AluOpType.bypass, op1=mybir.AluOpType.mult,
                    accum_out=qcol[:, t:t + 1])
            nc.vector.tensor_scalar(out=qcol, in0=qcol, scalar1=bq_bc, scalar2=None,
                                    op0=mybir.AluOpType.add)

            _, kk_bc = col_to_row(kcol, NT, "k")
            _, q_bc = col_to_row(qcol, NT, "q")

            # Pass 1 (orientation c-partition): exp + Z-accum; PE partial sums
            # over m < M_PE.
            imp_ps = pimp.tile([1, M_PE], F32)
            for t in range(NT):
                e_t = epool.tile([128, C], F32, tag="et")
                nc.scalar.activation(out=e_t, in_=kk_bc,
                                     func=mybir.ActivationFunctionType.Exp,
                                     bias=0.0, scale=qcol[:, t:t + 1],
                                     accum_out=zcol[:, t:t + 1])
                nc.vector.reciprocal(rcol[:, t:t + 1], zcol[:, t:t + 1])
                for (off, width) in PE_CHUNKS:
                    nc.tensor.matmul(
                        out=imp_ps[:, off:off + width],
                        lhsT=rcol[:, t:t + 1],
                        rhs=e_t[:, off:off + width],
                        start=(t == 0), stop=(t == NT - 1))

            # r broadcast for the DVE part.
            _, r_bc = col_to_row(rcol, NT, "r")

            # Pass 2 (orientation m-partition) for m in [M_PE, C): one ACT exp
            # + one DVE fused multiply-accum per 128-m tile.
            impT = qk.tile([128, NDVE], F32)
            for td in range(NDVE):
                et_t = epool.tile([128, C], F32, tag="et")
                nc.scalar.activation(out=et_t, in_=q_bc,
                                     func=mybir.ActivationFunctionType.Exp,
                                     bias=0.0,
                                     scale=kcol[:, M_PE // 128 + td:M_PE // 128 + td + 1])
                nc.vector.scalar_tensor_tensor(
                    out=dve_scratch, in0=et_t, scalar=0.0, in1=r_bc,
                    op0=mybir.AluOpType.bypass, op1=mybir.AluOpType.mult,
                    accum_out=impT[:, td:td + 1])

            # Assemble: [0, M_PE) from PSUM; [M_PE, C) from transposed impT.
            impT_ps = ptr.tile([NT, 128], F32, name="impT_ps", tag="tr")
            nc.tensor.transpose(out=impT_ps[:NDVE, :], in_=impT, identity=identity)
            impT_sb = small.tile([NT, 128], F32, name="impT_sb", tag="tsb")
            nc.vector.tensor_copy(impT_sb[:NDVE, :], impT_ps[:NDVE, :])
            imp_sb = small.tile([1, M_PE], F32, name="imp_sb", tag="imp_sb")
            nc.vector.tensor_copy(imp_sb, imp_ps)
            nc.sync.dma_start(out=imp.ap()[b:b + 1, 0:M_PE], in_=imp_sb)
            nc.sync.dma_start(
                out=imp.ap()[b:b + 1, M_PE:C].rearrange("o (g p) -> (o g) p", p=128),
                in_=impT_sb[:NDVE, :])
    nc.compile()
    return nc


def build_imp_kernel_v3(repeat=1, ndve=NDVE):
    """Software-pipelined: per iteration, emit (a) x-load + k/q projections +
    broadcasts for batch b+1, (b) transposed-pass colsums for batch b-1,
    (c) main exp/colsum pass for batch b — so ACT/DVE/PE queues never drain
    at batch boundaries."""
    m_pe = C - 128 * ndve
    chunks = []
    off = 0
    while off < m_pe:
        w = min(512, m_pe - off)
        chunks.append((off, w))
        off += w
    nc = bacc_mod.Bacc("TRN2", target_bir_lowering=False)
    x = nc.declare_dram_parameter("x", [BPC, C, D], F32, isOutput=False)
    wq = nc.declare_dram_parameter("wq", [D], F32, isOutput=False)
    wk = nc.declare_dram_parameter("wk", [D], F32, isOutput=False)
    bqk = nc.declare_dram_parameter("bqk", [2], F32, isOutput=False)
    imp = nc.declare_dram_parameter("imp", [BPC, C], F32, isOutput=True)

    NB = BPC * repeat

    with TileContext(nc) as tc, ExitStack() as ctx:
        const = ctx.enter_context(tc.tile_pool(name="const", bufs=1))
        xpool = ctx.enter_context(tc.tile_pool(name="xpool", bufs=2))
        epool = ctx.enter_context(tc.tile_pool(name="epool", bufs=5))
        bcpool = ctx.enter_context(tc.tile_pool(name="bcpool", bufs=2))
        qk = ctx.enter_context(tc.tile_pool(name="qk", bufs=2))
        small = ctx.enter_context(tc.tile_pool(name="small", bufs=2))
        big_scratch = ctx.enter_context(tc.tile_pool(name="bigs", bufs=1))
        pimp = ctx.enter_context(tc.tile_pool(name="pimp", bufs=2, space="PSUM"))
        ptr = ctx.enter_context(tc.tile_pool(name="ptr", bufs=2, space="PSUM"))

        def bcast_ap(handle, offset, n):
            a = handle.ap()
            return bass.AP(tensor=a.tensor, offset=offset, ap=[[0, 128], [1, n]])

        wq_bc = const.tile([128, D], F32)
        nc.gpsimd.dma_start(out=wq_bc, in_=bcast_ap(wq, 0, D))
        wk_bc = const.tile([128, D], F32)
        nc.gpsimd.dma_start(out=wk_bc, in_=bcast_ap(wk, 0, D))
        bq_bc = const.tile([128, 1], F32)
        nc.gpsimd.dma_start(out=bq_bc, in_=bcast_ap(bqk, 0, 1))
        bk_bc = const.tile([128, 1], F32)
        nc.gpsimd.dma_start(out=bk_bc, in_=bcast_ap(bqk, 1, 1))
        identity = const.tile([128, 128], F32)
        make_identity(nc, identity)

        dve_scratch = big_scratch.tile([128, C], F32)

        def col_to_row(col_ap, name, w_bc=None):
            t_ps = ptr.tile([NT, 128], F32, name=f"tps_{name}", tag="tr")
            nc.tensor.transpose(out=t_ps, in_=col_ap, identity=identity)
            t_sb = small.tile([NT, 128], F32, name=f"tsb_{name}", tag="tsb")
            nc.vector.tensor_copy(t_sb, t_ps)
            row = small.tile([1, C], F32, name=f"row_{name}", tag="row")
            nc.sync.dma_start(out=row, in_=t_sb)
            bc = bcpool.tile([128, C], F32, name=f"bc_{name}", tag=f"bc_{name}")
            nc.gpsimd.partition_broadcast(bc, row)
            return bc

        st = {}  # per-batch live state

        def stage_load_proj(nb):
            x_t = xpool.tile([128, NT, D], F32)
            half = NT // 2
            src = x.ap()[nb % BPC].rearrange("(t p) d -> p t d", p=128)
            nc.sync.dma_start(out=x_t[:, :half, :], in_=src[:, :half, :])
            nc.sync.dma_start(out=x_t[:, half:, :], in_=src[:, half:, :])
            qcol = qk.tile([128, NT], F32)
            kcol = qk.tile([128, NT], F32)
            scratch = qk.tile([128, D], F32)
            for t in range(NT):
                nc.vector.scalar_tensor_tensor(
                    out=scratch, in0=x_t[:, t, :], scalar=0.0, in1=wk_bc,
                    op0=mybir.AluOpType.bypass, op1=mybir.AluOpType.mult,
                    accum_out=kcol[:, t:t + 1])
            nc.vector.tensor_scalar(out=kcol, in0=kcol, scalar1=bk_bc, scalar2=None,
                                    op0=mybir.AluOpType.add)
            kk_bc = col_to_row(kcol, "k")
            for t in range(NT):
                nc.vector.scalar_tensor_tensor(
                    out=scratch, in0=x_t[:, t, :], scalar=0.0, in1=wq_bc,
                    op0=mybir.AluOpType.bypass, op1=mybir.AluOpType.mult,
                    accum_out=qcol[:, t:t + 1])
            nc.vector.tensor_scalar(out=qcol, in0=qcol, scalar1=bq_bc, scalar2=None,
                                    op0=mybir.AluOpType.add)
            q_bc = col_to_row(qcol, "q")
            st[nb] = dict(qcol=qcol, kcol=kcol, kk_bc=kk_bc, q_bc=q_bc)

        def stage_pass2(nb):
            s = st[nb]
            impT = qk.tile([128, max(ndve, 1)], F32)
            for td in range(ndve):
                et_t = epool.tile([128, C], F32, tag="et")
                nc.scalar.activation(out=et_t, in_=s["q_bc"],
                                     func=mybir.ActivationFunctionType.Exp,
                                     bias=0.0,
                                     scale=s["kcol"][:, m_pe // 128 + td:m_pe // 128 + td + 1])
                nc.vector.scalar_tensor_tensor(
                    out=dve_scratch, in0=et_t, scalar=0.0, in1=s["r_bc"],
                    op0=mybir.AluOpType.bypass, op1=mybir.AluOpType.mult,
                    accum_out=impT[:, td:td + 1])
            s["impT"] = impT

        def stage_pass1(nb):
            s = st[nb]
            zcol = qk.tile([128, NT], F32)
            rcol = qk.tile([128, NT], F32)
            imp_ps = pimp.tile([1, m_pe], F32)
            for t in range(NT):
                e_t = epool.tile([128, C], F32, tag="et")
                nc.scalar.activation(out=e_t, in_=s["kk_bc"],
                                     func=mybir.ActivationFunctionType.Exp,
                                     bias=0.0, scale=s["qcol"][:, t:t + 1],
                                     accum_out=zcol[:, t:t + 1])
                nc.vector.reciprocal(rcol[:, t:t + 1], zcol[:, t:t + 1])
                for (coff, w) in chunks:
                    nc.tensor.matmul(
                        out=imp_ps[:, coff:coff + w],
                        lhsT=rcol[:, t:t + 1],
                        rhs=e_t[:, coff:coff + w],
                        start=(t == 0), stop=(t == NT - 1))
            s["r_bc"] = col_to_row(rcol, "r")
            s["imp_ps"] = imp_ps

        def stage_finalize(nb):
            s = st.pop(nb)
            b = nb % BPC
            impT_ps = ptr.tile([NT, 128], F32, name="impT_ps", tag="tr")
            nc.tensor.transpose(out=impT_ps[:ndve, :], in_=s["impT"][:, :ndve],
                                identity=identity)
            impT_sb = small.tile([NT, 128], F32, name="impT_sb", tag="tsb")
            nc.vector.tensor_copy(impT_sb[:ndve, :], impT_ps[:ndve, :])
            imp_sb = small.tile([1, m_pe], F32, name="imp_sb", tag="imp_sb")
            nc.vector.tensor_copy(imp_sb, s["imp_ps"])
            nc.sync.dma_start(out=imp.ap()[b:b + 1, 0:m_pe], in_=imp_sb)
            nc.sync.dma_start(
                out=imp.ap()[b:b + 1, m_pe:C].rearrange("o (g p) -> (o g) p", p=128),
                in_=impT_sb[:ndve, :])

        def stage_pass2_tile(nb, td):
            s = st[nb]
            if td == 0:
                s["impT"] = qk.tile([128, max(ndve, 1)], F32, name="impT", tag="impT")
            et_t = epool.tile([128, C], F32, tag="et")
            nc.scalar.activation(out=et_t, in_=s["q_bc"],
                                 func=mybir.ActivationFunctionType.Exp,
                                 bias=0.0,
                                 scale=s["kcol"][:, m_pe // 128 + td:m_pe // 128 + td + 1])
            nc.vector.scalar_tensor_tensor(
                out=dve_scratch, in0=et_t, scalar=0.0, in1=s["r_bc"],
                op0=mybir.AluOpType.bypass, op1=mybir.AluOpType.mult,
                accum_out=s["impT"][:, td:td + 1])

        def stage_pass1_interleaved(nb):
            """pass1(nb) with pass2(nb-1) tiles woven into the tail so the
            ACT queue never drains at the batch boundary."""
            s = st[nb]
            zcol = qk.tile([128, NT], F32)
            rcol = qk.tile([128, NT], F32)
            imp_ps = pimp.tile([1, m_pe], F32)
            start_iv = NT - ndve
            for t in range(NT):
                if nb >= 1 and t >= start_iv:
                    stage_pass2_tile(nb - 1, t - start_iv)
                e_t = epool.tile([128, C], F32, tag="et")
                nc.scalar.activation(out=e_t, in_=s["kk_bc"],
                                     func=mybir.ActivationFunctionType.Exp,
                                     bias=0.0, scale=s["qcol"][:, t:t + 1],
                                     accum_out=zcol[:, t:t + 1])
                nc.vector.reciprocal(rcol[:, t:t + 1], zcol[:, t:t + 1])
                for (coff, w) in chunks:
                    nc.tensor.matmul(
                        out=imp_ps[:, coff:coff + w],
                        lhsT=rcol[:, t:t + 1],
                        rhs=e_t[:, coff:coff + w],
                        start=(t == 0), stop=(t == NT - 1))
            s["r_bc"] = col_to_row(rcol, "r")
            s["imp_ps"] = imp_ps

        stage_load_proj(0)
        for nb in range(NB):
            if nb + 1 < NB:
                stage_load_proj(nb + 1)
            stage_pass1_interleaved(nb)
            if nb >= 1:
                stage_finalize(nb - 1)
        for td in range(ndve):
            stage_pass2_tile(NB - 1, td)
        stage_finalize(NB - 1)
    nc.compile()
    return nc


def build_gather_kernel(repeat=1):
    nc = bacc_mod.Bacc("TRN2", target_bir_lowering=False)
    x = nc.declare_dram_parameter("x", [BPC, C, D], F32, isOutput=False)
    idx = nc.declare_dram_parameter("idx", [BPC, 128, KPAD // 16], I16, isOutput=False)
    sparse = nc.declare_dram_parameter("sparse", [BPC, KPAD, D], F32, isOutput=True)

    with TileContext(nc) as tc, ExitStack() as ctx:
        pool = ctx.enter_context(tc.tile_pool(name="g", bufs=2))
        for b in range(BPC * repeat):
            b = b % BPC
            idx_sb = pool.tile([128, KPAD // 16], I16)
            nc.sync.dma_start(out=idx_sb, in_=idx.ap()[b])
            g_sb = pool.tile([128, GD, D], F32)
            nc.gpsimd.dma_gather(
                out_ap=g_sb, in_ap=x.ap()[b], idxs_ap=idx_sb,
                num_idxs=KPAD, num_idxs_reg=KPAD, elem_size=D)
            nc.sync.dma_start(
                out=sparse.ap()[b].rearrange("(g p) d -> p g d", p=128),
                in_=g_sb)
    nc.compile()
    return nc


class PjrtRunner:
    """Cached jitted PJRT executor for one Bass module (mirrors
    bass2jax.run_bass_via_pjrt multi-core path, but reusable so repeat
    calls don't re-trace, enabling steady-state timing)."""

    def __init__(self, nc, n_cores=NCORES):
        import jax
        from jax.sharding import Mesh, PartitionSpec
        try:
            from jax.experimental.shard_map import shard_map
        except ImportError:  # newer jax
            from jax.shard_map import shard_map
        from concourse import bass2jax as b2j

        b2j.install_neuronx_cc_hook()
        self.jax = jax
        self.nc = nc
        self.n_cores = n_cores
        partition_name = (nc.partition_id_tensor.name
                          if nc.partition_id_tensor else None)
        in_names, out_names, out_avals, zero_outs = [], [], [], []
        for alloc in nc.m.functions[0].allocations:
            if not isinstance(alloc, mybir.MemoryLocationSet):
                continue
            name = alloc.memorylocations[0].name
            if alloc.kind == "ExternalInput":
                if name != partition_name and name != (
                        nc.dbg_addr.name if nc.dbg_addr else None):
                    in_names.append(name)
            elif alloc.kind == "ExternalOutput":
                shape = tuple(alloc.tensor_shape)
                dtype = mybir.dt.np(alloc.dtype)
                out_names.append(name)
                out_avals.append(jax.core.ShapedArray(shape, dtype))
                zero_outs.append(np.zeros(shape, dtype))
        self.in_names = list(in_names)
        self.out_names = out_names
        self.out_avals = out_avals
        self.zero_outs = zero_outs
        n_params = len(in_names)
        n_outs = len(out_names)
        all_in_names = list(in_names) + list(out_names)
        dbg_name = nc.dbg_addr.name if nc.dbg_addr else None
        if dbg_name is not None:
            all_in_names.append(dbg_name)
        if partition_name is not None:
            all_in_names.append(partition_name)
        self._dbg_name = dbg_name

        def _body(*args):
            operands = list(args)
            if dbg_name is not None:
                operands.append(jax.numpy.zeros((1, 2), np.uint32))
            if partition_name is not None:
                operands.append(b2j.partition_id_tensor())
            outs = b2j._bass_exec_p.bind(
                *operands,
                out_avals=tuple(out_avals),
                in_names=tuple(all_in_names),
                out_names=tuple(out_names),
                lowering_input_output_aliases=(),
                sim_require_finite=True,
                sim_require_nnan=True,
                nc=nc,
            )
            return tuple(outs)

        devices = jax.devices()[:n_cores]
        self.mesh = Mesh(np.asarray(devices), ("core",))
        in_specs = (PartitionSpec("core"),) * (n_params + n_outs)
        out_specs = (PartitionSpec("core"),) * n_outs
        self.fn = jax.jit(
            shard_map(_body, mesh=self.mesh, in_specs=in_specs,
                      out_specs=out_specs, check_rep=False),
            keep_unused=True,
        )

    def _concat_inputs(self, in_maps):
        return [np.concatenate([np.asarray(in_maps[c][n])
                                for c in range(self.n_cores)], axis=0)
                for n in self.in_names]

    def __call__(self, in_maps):
        concat_in = self._concat_inputs(in_maps)
        concat_zeros = [np.zeros((self.n_cores * z.shape[0], *z.shape[1:]),
                                 z.dtype) for z in self.zero_outs]
        outs = self.fn(*concat_in, *concat_zeros)
        return [
            {n: np.asarray(outs[i]).reshape(self.n_cores, *self.out_avals[i].shape)[c]
             for i, n in enumerate(self.out_names)}
            for c in range(self.n_cores)
        ]

    def device_args(self, in_maps):
        import jax
        from jax.sharding import NamedSharding, PartitionSpec
        sh = NamedSharding(self.mesh, PartitionSpec("core"))
        concat_in = self._concat_inputs(in_maps)
        concat_zeros = [np.zeros((self.n_cores * z.shape[0], *z.shape[1:]),
                                 z.dtype) for z in self.zero_outs]
        return [jax.device_put(a, sh) for a in concat_in + concat_zeros]

    def timed_ns(self, in_maps, iters=20, warmup=3):
        import time
        args = self.device_args(in_maps)
        for _ in range(warmup):
            outs = self.fn(*args)
            self.jax.block_until_ready(outs)
        ts = []
        for _ in range(iters):
            t0 = time.perf_counter()
            outs = self.fn(*args)
            self.jax.block_until_ready(outs)
            ts.append(time.perf_counter() - t0)
        return int(min(ts) * 1e9)

    def _chained_fn(self, m):
        """jit fn executing the bass program m times serially (call i's
        outputs feed call i+1's output-seed operands: data dep, no CSE)."""
        import jax
        from jax.sharding import PartitionSpec
        try:
            from jax.experimental.shard_map import shard_map
        except ImportError:
            from jax.shard_map import shard_map
        from concourse import bass2jax as b2j
        nc = self.nc
        n_params = len(self.in_names)
        n_outs = len(self.out_names)
        partition_name = (nc.partition_id_tensor.name
                          if nc.partition_id_tensor else None)
        all_in_names = list(self.in_names) + list(self.out_names)
        if self._dbg_name is not None:
            all_in_names.append(self._dbg_name)
        if partition_name is not None:
            all_in_names.append(partition_name)
        out_avals = tuple(self.out_avals)
        out_names = tuple(self.out_names)
        dbg_name = self._dbg_name

        def _body(*args):
            ins = list(args[:n_params])
            zouts = list(args[n_params:])
            for _ in range(m):
                operands = ins + zouts
                if dbg_name is not None:
                    operands.append(jax.numpy.zeros((1, 2), np.uint32))
                if partition_name is not None:
                    operands.append(b2j.partition_id_tensor())
                outs = b2j._bass_exec_p.bind(
                    *operands,
                    out_avals=out_avals,
                    in_names=tuple(all_in_names),
                    out_names=out_names,
                    lowering_input_output_aliases=(),
                    sim_require_finite=True,
                    sim_require_nnan=True,
                    nc=nc,
                )
                zouts = list(outs)
            return tuple(outs)

        in_specs = (PartitionSpec("core"),) * (n_params + n_outs)
        out_specs = (PartitionSpec("core"),) * n_outs
        return jax.jit(
            shard_map(_body, mesh=self.mesh, in_specs=in_specs,
                      out_specs=out_specs, check_rep=False),
            keep_unused=True,
        )

    def chained_timed_ns(self, in_maps, chain=8, iters=8, warmup=2):
        """Per-execution ns via (T(chain) - T(1)) / (chain - 1)."""
        import time
        args = self.device_args(in_maps)
        f1 = self._chained_fn(1)
        fm = self._chained_fn(chain)

        def best(fn):
            for _ in range(warmup):
                self.jax.block_until_ready(fn(*args))
            ts = []
            for _ in range(iters):
                t0 = time.perf_counter()
                self.jax.block_until_ready(fn(*args))
                ts.append(time.perf_counter() - t0)
            return min(ts)

        t1, tm = best(f1), best(fm)
        return int((tm - t1) / (chain - 1) * 1e9)


_CACHE = {}


_BUILDERS = {"imp": build_imp_kernel_v3, "gather": build_gather_kernel}


def _runner(key, repeat=1):
    ckey = f"{key}_r{repeat}"
    if ckey not in _CACHE:
        _CACHE[ckey] = PjrtRunner(_BUILDERS[key](repeat=repeat))
    return _CACHE[ckey]


def _in_maps_imp(x, Wq, Wk, bqk):
    return [{"x": x[c * BPC:(c + 1) * BPC], "wq": Wq, "wk": Wk, "bqk": bqk}
            for c in range(NCORES)]


def _in_maps_gather(x, idx16):
    return [{"x": x[c * BPC:(c + 1) * BPC], "idx": idx16[c * BPC:(c + 1) * BPC]}
            for c in range(NCORES)]


def _pack_idx16(topk_idx):
    # Flat rank order i lives at [i % 16, i // 16]; the 16-partition block is
    # replicated to all 8 Q7-core partition groups (each core reads its own).
    flat = np.zeros((B, KPAD), dtype=np.int16)
    flat[:, :K] = topk_idx
    blk = flat.reshape(B, KPAD // 16, 16).transpose(0, 2, 1)  # [B, 16, 40]
    return np.tile(blk, (1, 8, 1))  # [B, 128, 40]


def _median_call_ms(runner, in_maps, iters=12):
    import time
    import jax
    args = runner.device_args(in_maps)
    jax.block_until_ready(runner.fn(*args))
    ts = []
    for _ in range(iters):
        t0 = time.perf_counter()
        jax.block_until_ready(runner.fn(*args))
        ts.append((time.perf_counter() - t0) * 1e3)
    return float(np.median(ts))


def timed_hw_ns(inputs, rep=33, iters=16):
    """Per-execution device time of both launches via the repeat-variant
    differential: build each kernel at repeat=1 and repeat=rep (same I/O,
    rep x the work), time both with device-resident inputs, slope =
    (T(rep)-T(1))/(rep-1). Removes transfer/dispatch overhead, which
    dominates per-call wall time under the axon relay."""
    x = np.ascontiguousarray(np.asarray(inputs["x"], dtype=np.float32))
    Wq = np.asarray(inputs["Wq"], dtype=np.float32).reshape(-1)
    Wk = np.asarray(inputs["Wk"], dtype=np.float32).reshape(-1)
    bqk = np.array([np.asarray(inputs["bq"]).reshape(-1)[0],
                    np.asarray(inputs["bk"]).reshape(-1)[0]], dtype=np.float32)
    im = _in_maps_imp(x, Wq, Wk, bqk)
    t1 = _median_call_ms(_runner("imp", 1), im, iters)
    tr = _median_call_ms(_runner("imp", rep), im, iters)
    imp_ns = max(0.0, (tr - t1) / (rep - 1)) * 1e6
    res1 = _runner("imp", 1)(im)
    imp = np.concatenate([res1[c]["imp"] for c in range(NCORES)], axis=0)
    topk_idx = np.argsort(-imp, axis=-1, kind="stable")[:, :K].astype(np.int32)
    gm = _in_maps_gather(x, _pack_idx16(topk_idx))
    grep = max(rep, 65)  # gather is short; needs a larger multiple for signal
    g1 = _median_call_ms(_runner("gather", 1), gm, iters)
    gr = _median_call_ms(_runner("gather", grep), gm, iters)
    g_ns = max(0.0, (gr - g1) / (grep - 1)) * 1e6
    print(f"  launch1 (imp): {imp_ns:.0f} ns, launch2 (gather): {g_ns:.0f} ns")
    return int(imp_ns + g_ns)


def kernel(**inputs):
    x = np.ascontiguousarray(np.asarray(inputs["x"], dtype=np.float32))
    Wq = np.asarray(inputs["Wq"], dtype=np.float32).reshape(-1)
    Wk = np.asarray(inputs["Wk"], dtype=np.float32).reshape(-1)
    bq = np.asarray(inputs["bq"], dtype=np.float32).reshape(-1)
    bk = np.asarray(inputs["bk"], dtype=np.float32).reshape(-1)
    assert x.shape == (B, C, D)

    core_ids = list(range(NCORES))
    bqk = np.array([bq[0], bk[0]], dtype=np.float32)
    res1 = _runner("imp")(_in_maps_imp(x, Wq, Wk, bqk))
    imp = np.concatenate([res1[c]["imp"] for c in core_ids], axis=0)

    # Top-k on host: stable argsort desc == jax.lax.top_k ordering.
    topk_idx = np.argsort(-imp, axis=-1, kind="stable")[:, :K].astype(np.int32)

    # Pack indices for dma_gather: flat rank order i lives at [i % 16, i // 16]
    # of the first 16 partitions; pad ranks K..KPAD with 0 (extra rows are
    # gathered but sliced off on the host).
    idx16 = _pack_idx16(topk_idx)
    res2 = _runner("gather")(_in_maps_gather(x, idx16))
    sparse = np.concatenate(
        [res2[c]["sparse"][:, :K, :] for c in core_ids], axis=0)

    return sparse, topk_idx, K
